# revision 22
# baseline (speedup 1.0000x reference)
"""Trainium2 Bass kernel for the Channelenhance (topk channel masking) module.

Computation (per sample n):
    z = mean(x[n], axis=(H,W))                  # (256,)
    h = relu(W1 @ z + b1)                       # (128,)
    u = W2 @ h + b2                             # (256,) pre-sigmoid logits
    order = argsort(-sigmoid(u)) = argsort(-u)  # sigmoid is monotone
    selected  = x[n, order[:128]]
    remaining = x[n, order[128:]]

Strategy: pure data parallel over N=16 samples on 8 cores (2 samples/core).
Each sample's x (16 MB) is DMA'd into SBUF once, the ranks are computed on
device (GAP via two-stage DVE reductions, the tiny MLP via exact fp32 DVE
ops with PE-transpose broadcasts), and the channels are written back with
indirect (scatter) DMA directly into their sorted positions.  Total HBM
traffic = read x once + write output once (the memory-roofline minimum).

rank[c] = #{j: u[j] > u[c]} + #{j < c: u[j] == u[c]} is the stable
descending argsort inverse, so channel c's row goes to output row rank[c]
of the (sample, spatial-half) output tensor.

Built on bacc.Bacc (not raw bass.Bass): walrus codegen allows only one
sync wait per instruction on TRN2 and Bacc's split_sync_waits /
event-semaphore passes legalize Tile's multi-wait instructions.
"""

import os
from contextlib import ExitStack

import numpy as np

import concourse.bacc as bacc
import concourse.bass as bass
import concourse.mybir as mybir
from concourse.bass import IndirectOffsetOnAxis
from concourse.bass_utils import run_bass_kernel_spmd
from concourse.masks import make_identity
from concourse.tile import TileContext

P = 128          # partitions / channel block
C = 256          # channels
HW = 128 * 128   # spatial size per channel
HALF = HW // 2   # elements per scatter row
NS = 2           # samples per core
NCORES = 8

F32 = mybir.dt.float32
I32 = mybir.dt.int32
AX = mybir.AxisListType
OP = mybir.AluOpType

# const blob column layout
CB_W1 = 0        # (128, 256)  W1 / 2^14
CB_W2 = 256      # (128, 256)  col block b = W2[b*128:(b+1)*128, :]
CB_MLT = 512     # (128, 512)  col range b*256.. = "j < i" mask rows, block b
CB_B1 = 1024     # (128, 1)
CB_B2 = 1025     # (128, 2)    col b = b2[b*128:(b+1)*128]
CB_COLS = 1027

LAST_RESULTS = None  # test harness introspection


def _build_program(W1, b1, W2, b2):
    nc = bacc.Bacc()

    xs = nc.dram_tensor("xs", [NS * C, HW], F32, kind="ExternalInput")
    outs_d = {
        (n, h): nc.dram_tensor(f"out{n}{h}", [C, HALF], F32, kind="ExternalOutput")
        for n in range(NS) for h in range(2)
    }
    # Host-side packed const blob.  1/HW is folded into W1 (HW = 2^14, so
    # the scaling is exact and W1s @ colsum == W1 @ mean bit-for-bit).
    W1 = np.asarray(W1, np.float32)
    W2 = np.asarray(W2, np.float32)
    b1 = np.asarray(b1, np.float32)
    b2 = np.asarray(b2, np.float32)
    blob = np.zeros((P, CB_COLS), np.float32)
    blob[:, CB_W1:CB_W1 + C] = W1 / np.float32(HW)
    blob[:, CB_W2:CB_W2 + C] = np.concatenate([W2[:P, :], W2[P:, :]], axis=1)
    j_idx = np.arange(C, dtype=np.int64)
    for b in range(2):
        i_glob = b * P + np.arange(P)[:, None]
        blob[:, CB_MLT + b * C: CB_MLT + (b + 1) * C] = (
            j_idx[None, :] < i_glob).astype(np.float32)
    blob[:, CB_B1] = b1
    blob[:, CB_B2:CB_B2 + 2] = b2.reshape(2, P).T
    blob_d = nc.inline_tensor(np.ascontiguousarray(blob), name="cblob")

    with TileContext(nc) as tc, ExitStack() as ctx:
        consts = ctx.enter_context(tc.tile_pool(name="consts", bufs=1))
        xpool = ctx.enter_context(tc.tile_pool(name="xt", bufs=2))
        small = ctx.enter_context(tc.tile_pool(name="small", bufs=2))
        cmp_p = ctx.enter_context(tc.tile_pool(name="cmp", bufs=2))
        idx_p = ctx.enter_context(tc.tile_pool(name="idx", bufs=2))
        psum = ctx.enter_context(tc.tile_pool(name="ps", bufs=3, space="PSUM"))

        ident = consts.tile([P, P], F32)
        make_identity(nc, ident)
        cb = consts.tile([P, CB_COLS], F32)
        nc.sync.dma_start(out=cb[:], in_=blob_d[:])
        w1s = cb[:, CB_W1:CB_W1 + C]
        w2b = cb[:, CB_W2:CB_W2 + C]
        mlt = cb[:, CB_MLT:CB_MLT + 2 * C]
        b1c = cb[:, CB_B1:CB_B1 + 1]
        b2c = cb[:, CB_B2:CB_B2 + 2]

        zero1 = consts.tile([P, 1], F32)
        nc.vector.memset(zero1[:], 0.0)

        # Warm-up transpose: absorbs PE's one-time wait on the gpsimd-built
        # identity so each per-sample transpose needs only its DVE wait.
        warm = psum.tile([P, P], F32, tag="warm", space="PSUM", bufs=1)
        nc.tensor.transpose(out=warm[:], in_=ident[:], identity=ident[:])

        xs_ap = xs[:]
        for n in range(NS):
            # ---- load x[n] as two (128, [block0 | block1]) half tiles ----
            xt = {}
            stats = small.tile([P, 4], F32, tag="stats")  # col = 2*b + h
            for h in range(2):
                t = xpool.tile([P, 2 * HALF], F32, tag="xt")
                xt[h] = t
                src = xs_ap[n * C:(n + 1) * C, h * HALF:(h + 1) * HALF]
                nc.sync.dma_start(
                    out=t[:].rearrange("p (j c) -> p j c", c=HALF),
                    in_=src.rearrange("(j p) c -> p j c", p=P),
                )
            # ---- GAP partial sums, two-stage for fp32 accuracy ----
            for h in range(2):
                for b in range(2):
                    tmp = small.tile([P, 64], F32, tag="tmp", bufs=8)
                    nc.vector.tensor_reduce(
                        out=tmp[:],
                        in_=xt[h][:, b * HALF:(b + 1) * HALF].rearrange(
                            "p (a q) -> p a q", q=P),
                        axis=AX.X, op=OP.add,
                    )
                    nc.vector.tensor_reduce(
                        out=stats[:, 2 * b + h:2 * b + h + 1], in_=tmp[:],
                        axis=AX.X, op=OP.add,
                    )
            z2 = small.tile([P, 2], F32, tag="z2")
            for b in range(2):
                nc.vector.tensor_reduce(
                    out=z2[:, b:b + 1], in_=stats[:, 2 * b:2 * b + 2],
                    axis=AX.X, op=OP.add,
                )

            # ---- broadcast z blocks across partitions via PE transpose ----
            zbs = []
            for b in range(2):
                zp = psum.tile([P, P], F32, tag="zb", space="PSUM")
                nc.tensor.transpose(
                    out=zp[:], in_=z2[:, b:b + 1].to_broadcast([P, P]),
                    identity=ident[:],
                )
                zbs.append(zp)

            # ---- h = relu(W1s @ zsum + b1), staged fp32 reductions ----
            prod = cmp_p.tile([P, C], F32, tag="prod")
            for b in range(2):
                nc.vector.tensor_tensor(
                    out=prod[:, b * P:(b + 1) * P], in0=w1s[:, b * P:(b + 1) * P],
                    in1=zbs[b][:], op=OP.mult,
                )
            h8 = small.tile([P, 8], F32, tag="h8")
            nc.vector.tensor_reduce(
                out=h8[:], in_=prod[:].rearrange("p (a q) -> p a q", q=32),
                axis=AX.X, op=OP.add,
            )
            hraw = small.tile([P, 1], F32, tag="hraw")
            nc.vector.tensor_reduce(out=hraw[:], in_=h8[:], axis=AX.X, op=OP.add)
            hcol = small.tile([P, 1], F32, tag="hcol")
            nc.vector.scalar_tensor_tensor(
                out=hcol[:], in0=hraw[:], scalar=b1c, in1=zero1[:],
                op0=OP.add, op1=OP.max,
            )
            hb = psum.tile([P, P], F32, tag="zb", space="PSUM")
            nc.tensor.transpose(
                out=hb[:], in_=hcol[:].to_broadcast([P, P]), identity=ident[:]
            )

            # ---- u = W2 @ h + b2 per channel block ----
            u2 = small.tile([P, 2], F32, tag="u2")
            for b in range(2):
                produ = cmp_p.tile([P, P], F32, tag="produ")
                nc.vector.tensor_tensor(
                    out=produ[:], in0=w2b[:, b * P:(b + 1) * P], in1=hb[:],
                    op=OP.mult,
                )
                u4 = small.tile([P, 4], F32, tag="u4")
                nc.vector.tensor_reduce(
                    out=u4[:], in_=produ[:].rearrange("p (a q) -> p a q", q=32),
                    axis=AX.X, op=OP.add,
                )
                uraw = small.tile([P, 1], F32, tag="uraw")
                nc.vector.tensor_reduce(out=uraw[:], in_=u4[:], axis=AX.X, op=OP.add)
                nc.vector.tensor_tensor(
                    out=u2[:, b:b + 1], in0=uraw[:], in1=b2c[:, b:b + 1], op=OP.add
                )

            # ---- broadcast u across partitions (each row = full u block) ----
            ubs = []
            for b in range(2):
                up = psum.tile([P, P], F32, tag="ub", space="PSUM")
                nc.tensor.transpose(
                    out=up[:], in_=u2[:, b:b + 1].to_broadcast([P, P]),
                    identity=ident[:],
                )
                ubs.append(up)

            # ---- stable descending ranks ----
            ranks = idx_p.tile([P, 2], F32, tag="ranks")  # col b = rank of block b
            for b in range(2):
                accs_g, accs_e = [], []
                for jb in range(2):
                    cmpo = cmp_p.tile([P, P], F32, tag="cmpo")
                    acc_g = small.tile([P, 1], F32, tag="accg")
                    nc.vector.tensor_scalar(
                        out=cmpo[:], in0=ubs[jb][:], scalar1=u2[:, b:b + 1],
                        scalar2=None, op0=OP.is_gt, op1=OP.add, accum_out=acc_g[:],
                    )
                    accs_g.append(acc_g)
                    cmpe = cmp_p.tile([P, P], F32, tag="cmpo")
                    acc_e = small.tile([P, 1], F32, tag="acce")
                    m = mlt[:, b * C + jb * P: b * C + (jb + 1) * P]
                    nc.vector.scalar_tensor_tensor(
                        out=cmpe[:], in0=ubs[jb][:], scalar=u2[:, b:b + 1], in1=m,
                        op0=OP.is_equal, op1=OP.mult, accum_out=acc_e[:],
                    )
                    accs_e.append(acc_e)
                s1 = small.tile([P, 1], F32, tag="s1")
                nc.vector.tensor_tensor(out=s1[:], in0=accs_g[0][:], in1=accs_g[1][:], op=OP.add)
                s2 = small.tile([P, 1], F32, tag="s2")
                nc.vector.tensor_tensor(out=s2[:], in0=accs_e[0][:], in1=accs_e[1][:], op=OP.add)
                nc.vector.tensor_tensor(
                    out=ranks[:, b:b + 1], in0=s1[:], in1=s2[:], op=OP.add
                )

            idxi = idx_p.tile([P, 2], I32, tag="idxi")
            nc.vector.tensor_copy(out=idxi[:], in_=ranks[:])

            # ---- scatter: channel (j*128+p)'s half h -> row rank[j*128+p] ----
            # One scatter per (half, block): the offset AP must be one index
            # per partition — multi-column offset APs scatter incorrectly on
            # hardware (CoreSim models them fine; HW does not).
            for h in range(2):
                for j in range(2):
                    nc.gpsimd.indirect_dma_start(
                        out=outs_d[n, h][:],
                        out_offset=IndirectOffsetOnAxis(ap=idxi[:, j:j + 1], axis=0),
                        in_=xt[h][:, j * HALF:(j + 1) * HALF],
                        in_offset=None,
                    )
    return nc


def _run(x, W1, b1, W2, b2, trace=False, **trace_kwargs):
    x = np.ascontiguousarray(np.asarray(x, np.float32))
    nc = _build_program(W1, b1, W2, b2)
    nc.finalize()  # runs Bacc's reg-alloc + sync-wait legalization passes
    in_maps = [
        {"xs": np.ascontiguousarray(x[NS * c:NS * (c + 1)].reshape(NS * C, HW))}
        for c in range(NCORES)
    ]
    res = run_bass_kernel_spmd(nc, in_maps, list(range(NCORES)), trace=trace, **trace_kwargs)
    return res


def _assemble(res):
    N = NS * NCORES
    sel = np.empty((N, P, 128, 128), np.float32)
    rem = np.empty((N, P, 128, 128), np.float32)
    o = np.empty((C, HW), np.float32)
    for c in range(NCORES):
        for n in range(NS):
            o[:, :HALF] = np.asarray(res.results[c][f"out{n}0"])
            o[:, HALF:] = np.asarray(res.results[c][f"out{n}1"])
            v = o.reshape(C, 128, 128)
            sel[NS * c + n] = v[:P]
            rem[NS * c + n] = v[P:]
    return sel, rem


def kernel(x, W1, b1, W2, b2):
    global LAST_RESULTS
    res = _run(x, W1, b1, W2, b2, trace=bool(os.environ.get("BASS_TRACE")))
    LAST_RESULTS = res
    return _assemble(res)
